# revision 23
# baseline (speedup 1.0000x reference)
"""Linear-attention (sparse_attention) Trainium2 Bass kernel.

Problem: nn_Attention_Linear_25709674234652
  B=4, S=8192, D=1024, H=16 heads, HD=64, AD=64 (approx dim), EPS=1e-6

  qkv = x @ W_qkv.T (+0)          [B,S,3D]
  per head: pQ = Q @ W_p.T, pK = K @ W_p.T, phi(u) = sqrt(1+u^2)
  KTV = phi_K^T @ V  [AD,HD],  k_sum = sum_s phi_K
  out = (phi_Q @ KTV) / (phi_Q @ k_sum + eps)

Sharding: 8 cores = 4 batches x 2 head-groups (8 heads each). Each core is
fully independent (no collectives).

Host-side tricks:
  - W_p @ W_q and W_p @ W_k are folded into single projection matrices, so
    the device computes pQ / pK directly from x; Q and K never exist.
  - x is passed transposed (x^T) so the contraction dim D is already on
    partitions; no on-chip transposes anywhere.
  - Q/K projections run in fp8-e4m3 DoubleRow (2x PE rate): their
    quantization errors largely cancel in the num/den ratio (phi damping +
    shared-error cancellation), host-sim rel err ~5.5e-3. The V projection
    must stay bf16: V-path errors hit the output un-averaged (x-fp8 for V
    alone costs ~2.1e-2).
  - output shipped bf16 and upcast to f32 on host (halves the out-DMA
    tail in pass B).

Device structure:
  - pass A (per 512-col s-block): pQ^T feature-major via fp8 DoubleRow
    (4 k-pair matmuls) -> phi -> bf16 phi_Q kept RESIDENT in SBUF; pK
    row-major fp8 DoubleRow + V row-major bf16 -> phi(pK), V -> KTV
    accumulated over all of S in PSUM (k_sum rides along as a
    ones-column appended to V). KTV matmuls are emitted ~3 blocks late
    so the in-order PE never waits on the ACT phi chain.
  - pass B (per 128-row s-block): one N=130 matmul per head-pair against
    block-diagonal KTV + k_sum columns (num and den in one shot),
    reciprocal + broadcast multiply on DVE, store bf16. The last QSHIFT
    s-blocks' pQ matmul groups are deferred into pass B to fill its
    otherwise-idle PE (they have no KTV dependency).
"""

import numpy as np
import ml_dtypes

import concourse.bass as bass
import concourse.tile as tile
from concourse import bacc, mybir
from concourse.bass_utils import run_bass_kernel_spmd

# ---- problem dims (hardcoded per spec) ----
B, S, D = 4, 8192, 1024
H, HD, AD = 16, 64, 64
EPS = 1e-6
NCORES = 8
HG = H // 2          # heads per core = 8
CH = HG * AD         # phi channels per core = 512
CV = HG * HD         # value channels per core = 512
P = 128
NKD = D // P         # 8 contraction tiles
NKP = NKD // 2       # 4 fp8 DoubleRow contraction pair-tiles
SB = 512             # pass-A s-block
NSB = S // SB        # 16
NPAIR = CH // P      # 4 head-pairs per core
NB2 = S // P         # 64 pass-B s-blocks
F32 = mybir.dt.float32
BF16 = mybir.dt.bfloat16
F8 = mybir.dt.float8e4
DR = mybir.MatmulPerfMode.DoubleRow

_CACHE = {}
LAST_RESULTS = None  # BassKernelResults of most recent run (for profiling)


def _build_nc():
    nc = bacc.Bacc()
    AF = mybir.ActivationFunctionType

    xt8 = nc.dram_tensor("xt8", [D, S], F8, kind="ExternalInput")
    xbf = nc.dram_tensor("xbf", [D, S], BF16, kind="ExternalInput")
    wq8 = nc.dram_tensor("wq8", [D, CH], F8, kind="ExternalInput")
    wk8 = nc.dram_tensor("wk8", [D, CH], F8, kind="ExternalInput")
    wv = nc.dram_tensor("wv", [D, CV], BF16, kind="ExternalInput")
    out = nc.dram_tensor("out", [S, CV], BF16, kind="ExternalOutput")

    xt8_r = xt8.rearrange("(kd p) s -> p kd s", p=P)
    xbf_r = xbf.rearrange("(kd p) s -> p kd s", p=P)
    wq8_r = wq8.rearrange("(kd p) c -> p kd c", p=P)
    wk8_r = wk8.rearrange("(kd p) c -> p kd c", p=P)
    wv_r = wv.rearrange("(kd p) c -> p kd c", p=P)

    with tile.TileContext(nc) as tc:
        with (
            tc.tile_pool(name="singles", bufs=1) as singles,
            tc.tile_pool(name="x8load", bufs=2) as x8load,
            tc.tile_pool(name="xbload", bufs=2) as xbload,
            tc.tile_pool(name="sqp", bufs=3) as sqpool,
            tc.tile_pool(name="phikp", bufs=6) as phikpool,
            tc.tile_pool(name="vp", bufs=6) as vpool,
        ):
            # startup critical path: per-kd DMAs so the first matmul starts
            # after a few hundred KB, not the full working set
            def load_x_block(sb, persist=False):
                """fp8 pair tiles [P, 2, SB] (t=kd pair) + bf16 tiles [P, SB].

                persist: deferred-pQ blocks keep their fp8 tiles resident in
                SBUF until pass B consumes them (no re-DMA, no LDW waits)."""
                t8, tbf = [], []
                for t in range(NKP):
                    if persist:
                        x8_t = x8load.tile([P, 2, SB], F8, tag=f"xqp_{sb}_{t}",
                                           name=f"x8_{sb}_{t}", bufs=1)
                    else:
                        x8_t = x8load.tile([P, 2, SB], F8, tag=f"x8_{t}",
                                           name=f"x8_{sb}_{t}")
                    for i in range(2):
                        nc.sync.dma_start(
                            out=x8_t[:, i, :],
                            in_=xt8_r[:, 2 * t + i, sb * SB:(sb + 1) * SB],
                        )
                    t8.append(x8_t)
                for kd in range(NKD):
                    xb_t = xbload.tile([P, SB], BF16, tag=f"xb{kd}",
                                       name=f"xb_{sb}_{kd}")
                    nc.sync.dma_start(
                        out=xb_t, in_=xbf_r[:, kd, sb * SB:(sb + 1) * SB]
                    )
                    tbf.append(xb_t)
                return t8, tbf

            # startup DMA order = dependency order of the early PE work:
            # x8(sb0)+wq8 (pQ sb0) -> x8(sb1) (pQ sb1) -> wk8 (pK) ->
            # xbf(sb0)+wv (V sb0) -> xbf(sb1). The two pQ emissions give the
            # PE ~14us of work before V's operands are needed, so no warm
            # bridges are required.
            w_q8 = singles.tile([P, NKD, CH], F8)
            w_k8 = singles.tile([P, NKD, CH], F8)
            w_v = singles.tile([P, NKD, CV], BF16)
            x8_first, xbf_first = [], []
            x8_second, xbf_second = [], []
            for t in range(NKP):
                x8_t = x8load.tile([P, 2, SB], F8, tag=f"x8_{t}",
                                   name=f"x8_0_{t}")
                for i in range(2):
                    nc.sync.dma_start(out=x8_t[:, i, :],
                                      in_=xt8_r[:, 2 * t + i, 0:SB])
                    nc.sync.dma_start(out=w_q8[:, 2 * t + i],
                                      in_=wq8_r[:, 2 * t + i])
                x8_first.append(x8_t)
            for t in range(NKP):
                x8_t = x8load.tile([P, 2, SB], F8, tag=f"x8_{t}",
                                   name=f"x8_1_{t}")
                for i in range(2):
                    nc.sync.dma_start(out=x8_t[:, i, :],
                                      in_=xt8_r[:, 2 * t + i, SB:2 * SB])
                x8_second.append(x8_t)
            for kd in range(NKD):
                nc.sync.dma_start(out=w_k8[:, kd], in_=wk8_r[:, kd])
            for kd in range(NKD):
                xb_t = xbload.tile([P, SB], BF16, tag=f"xb{kd}",
                                   name=f"xb_0_{kd}")
                nc.sync.dma_start(out=xb_t, in_=xbf_r[:, kd, 0:SB])
                xbf_first.append(xb_t)
                nc.sync.dma_start(out=w_v[:, kd], in_=wv_r[:, kd])
            for kd in range(NKD):
                xb_t = xbload.tile([P, SB], BF16, tag=f"xb{kd}",
                                   name=f"xb_1_{kd}")
                nc.sync.dma_start(out=xb_t, in_=xbf_r[:, kd, SB:2 * SB])
                xbf_second.append(xb_t)
            # phi_Q^T resident: [128, 4 q-tiles, S] bf16 = 64 KiB/partition
            phiq_sb = singles.tile([P, NPAIR, S], BF16)

            with (
                tc.tile_pool(name="ps_q", bufs=2, space="PSUM") as ps_q,
                tc.tile_pool(name="ps_k", bufs=2, space="PSUM") as ps_k,
                tc.tile_pool(name="ps_v", bufs=2, space="PSUM") as ps_v,
                tc.tile_pool(name="ps_acc", bufs=1, space="PSUM") as ps_acc,
            ):
                # startup: warm matmuls on DVE-memset tiles (no DMA dep)
                # fill the initial DMA wait and spin HAM up to 2.4 GHz
                # before the first real matmul
                warm_a = singles.tile([P, P], BF16)
                nc.vector.memset(warm_a, 0.5)
                warm_b = singles.tile([P, SB], BF16)
                nc.vector.memset(warm_b, 0.5)
                wp0 = ps_q.tile([P, SB], F32, tag="pq", name="warm_start")
                for k in range(12):
                    nc.tensor.matmul(
                        wp0, warm_a, warm_b, start=(k == 0), stop=(k == 11)
                    )

                def warm_bridge(n, key):
                    # no-dep filler MMs to bridge startup DMA stalls so HAM
                    # never sees an idle window while the x/w streams land
                    wp = ps_q.tile([P, SB], F32, tag="pq", name=f"wb_{key}")
                    for k in range(n):
                        nc.tensor.matmul(
                            wp, warm_a, warm_b, start=(k == 0), stop=(k == n - 1)
                        )

                # persistent accumulators, live across the whole pass.
                # col 128 of each pair block accumulates k_sum (ones column
                # appended to V), so no separate ksum matmuls are needed.
                PV1 = P + 1
                ktv_ps_ab = [
                    ps_acc.tile([P, 2, PV1], F32, tag=f"ktv{i}", name=f"ktv{i}")
                    for i in range(2)
                ]

                pending = []

                def emit_ktv(phik_t, v_t, idx):
                    first = idx == 0
                    last = idx == 4 * NSB - 1
                    for pr in range(NPAIR):
                        # [128s x 128a].T @ [128s x 129(v|1)] -> a-pair x (v|ksum)
                        # off-diagonal 64x64 blocks are cross-head garbage,
                        # masked out when copying to SBUF.
                        nc.tensor.matmul(
                            ktv_ps_ab[pr // 2][:, pr % 2, :],
                            phik_t[:, pr * P:(pr + 1) * P],
                            v_t[:, pr, :],
                            start=(first and pr % 2 == 0),
                            stop=(last and pr % 2 == 1),
                        )

                def emit_pq_qt(x8_t, sb, qt, pool):
                    # one pQ^T q-tile: fp8 DoubleRow matmul group + phi ->
                    # resident bf16
                    pq_ps = pool.tile([P, SB], F32, tag="pq",
                                      name=f"pq_{sb}_{qt}")
                    for t in range(NKP):
                        nc.tensor.matmul(
                            pq_ps,
                            w_q8[:, 2 * t:2 * t + 2, qt * P:(qt + 1) * P],
                            x8_t[t],
                            start=(t == 0),
                            stop=(t == NKP - 1),
                            perf_mode=DR,
                        )
                    sq_t = sqpool.tile([P, SB], F32, tag="sq_q")
                    nc.scalar.square(sq_t, pq_ps)
                    nc.scalar.activation(
                        phiq_sb[:, qt, sb * SB:(sb + 1) * SB],
                        sq_t, AF.Sqrt, bias=1.0,
                    )

                def emit_pq(x8_t, sb, pool):
                    for qt in range(NPAIR):
                        emit_pq_qt(x8_t, sb, qt, pool)

                # the last QSHIFT blocks' pQ groups are deferred into pass B
                # (no KTV dependency): spread over pass B at qt-group
                # granularity to keep the PE dense there
                QSHIFT = 11
                QS0 = NSB - QSHIFT
                xq_blocks = {}
                # pQ of sb0 AND sb1 first: their deps (x8 + wq8, ~1.7 MiB)
                # land long before V's xbf/wv, keeping the PE busy through
                # the startup DMA window
                emit_pq(x8_first, 0, ps_q)
                emit_pq(x8_second, 1, ps_q)
                # startup is DMA-throughput-bound (~20us to land xbf+wv);
                # DMA-free filler keeps HAM at 2.4 GHz until V's deps arrive
                warm_bridge(6, "post_pq")
                for sb in range(NSB):
                    if sb == 0:
                        x8_t, xbf_t = x8_first, xbf_first
                    elif sb == 1:
                        x8_t, xbf_t = x8_second, xbf_second
                    else:
                        x8_t, xbf_t = load_x_block(sb, persist=(sb >= QS0))
                    if 2 <= sb < QS0:
                        emit_pq(x8_t, sb, ps_q)
                    elif sb >= QS0:
                        xq_blocks[sb - QS0] = x8_t
                    # ---- row-major pK (fp8 DR) | V (bf16) + phi + KTV ----
                    for st in range(4):
                        pk_ps = ps_k.tile([P, CH], F32, tag="pk")
                        v_ps = ps_v.tile([P, CV], F32, tag="v")
                        for t in range(NKP):
                            nc.tensor.matmul(
                                pk_ps,
                                x8_t[t][:, :, st * P:(st + 1) * P],
                                w_k8[:, 2 * t:2 * t + 2, :],
                                start=(t == 0), stop=(t == NKP - 1),
                                perf_mode=DR,
                            )
                        if sb == 0:
                            warm_bridge((3, 2, 1, 1)[st], f"s0_{st}")
                        elif sb == 1 and st == 0:
                            warm_bridge(1, "s1_0")
                        # emit deferred KTV here: the tiny KTV matmuls land
                        # between the DR pk group and the bf16 V group, so
                        # the next DR LDWEIGHTS never trails a tiny-MM burst
                        while len(pending) > 3:
                            emit_ktv(*pending.pop(0))
                        for kd in range(NKD):
                            nc.tensor.matmul(
                                v_ps,
                                xbf_t[kd][:, st * P:(st + 1) * P],
                                w_v[:, kd, :],
                                start=(kd == 0), stop=(kd == NKD - 1),
                            )
                        sqk_t = sqpool.tile([P, CH], F32, tag="sq_k")
                        nc.scalar.square(sqk_t, pk_ps)
                        phik_t = phikpool.tile([P, CH], BF16, tag="phik")
                        nc.scalar.activation(phik_t, sqk_t, AF.Sqrt, bias=1.0)
                        # V pairs with a ones column appended (k_sum rides the
                        # KTV matmul as output column 128)
                        v_t = vpool.tile([P, NPAIR, P + 1], BF16, tag="vsb")
                        nc.vector.tensor_copy(
                            out=v_t[:, :, 0:P],
                            in_=v_ps[:, :].rearrange("p (q v) -> p q v", v=P),
                        )
                        nc.vector.memset(v_t[:, :, P:P + 1], 1.0)
                        pending.append((phik_t, v_t, sb * 4 + st))
                for item in pending:
                    emit_ktv(*item)
                pending.clear()

                # ---- KTV -> block-diag SBUF (bf16), ksum in cols 128-129 ----
                # rhs_all[:, pr] = [ktv_bd (128) | ksum_h0 col | ksum_h1 col]
                # so pass B's den rides the same matmul as num (N=130).
                rhs_all = singles.tile([P, NPAIR, P + 2], BF16)
                nc.vector.memset(rhs_all, 0.0)
                HA = AD  # 64
                for pr in range(NPAIR):
                    kps = ktv_ps_ab[pr // 2][:, pr % 2, :]
                    nc.vector.tensor_copy(
                        out=rhs_all[0:HA, pr, 0:HA], in_=kps[0:HA, 0:HA]
                    )
                    nc.vector.tensor_copy(
                        out=rhs_all[HA:P, pr, HA:P], in_=kps[HA:P, HA:P]
                    )
                    nc.vector.tensor_copy(
                        out=rhs_all[0:HA, pr, P:P + 1], in_=kps[0:HA, P:P + 1]
                    )
                    nc.vector.tensor_copy(
                        out=rhs_all[HA:P, pr, P + 1:P + 2], in_=kps[HA:P, P:P + 1]
                    )

            # ---- pass B: numerator / denominator / divide / store ----
            with (
                tc.tile_pool(name="ps_nd", bufs=3, space="PSUM") as ps_nd,
                tc.tile_pool(name="ps_q2", bufs=2, space="PSUM") as ps_q2,
                tc.tile_pool(name="bwork", bufs=4) as bwork,
                tc.tile_pool(name="bout", bufs=4) as bout,
            ):
                NDW = P + 2  # num (128) + den (2) columns per pair
                NQG = NPAIR * QSHIFT   # deferred qt-groups
                # uniform deadline-aware placement: group g at use_block(g),
                # spread over nearly all of pass B so the PE (and HAM) stay
                # dense to the end. Deadline: group (j, qt) must land >=2
                # blocks before nd block 4*(QS0+j) reads its phiq. The x8
                # operands are already resident in SBUF (persist=True above).
                use_block = {}
                for g in range(NQG):
                    j = g // NPAIR
                    blk = 1 + (g * 58) // max(NQG, 1)
                    blk = min(blk, 4 * (QS0 + j) - 2)
                    use_block[g] = blk
                groups_at = {}
                for g in range(NQG):
                    groups_at.setdefault(use_block[g], []).append(g)

                def emit_warm_mm(n, key):
                    # dummy matmuls on resident weights into a dead psum tile:
                    # keeps the PE duty cycle high enough that HAM doesn't
                    # re-throttle to 1.2 GHz during bursty stretches
                    wp = ps_q2.tile([P, SB], F32, tag="pq", name=f"warm_{key}")
                    for k in range(n):
                        nc.tensor.matmul(
                            wp, w_v[:, k, 0:P], w_v[:, k, :],
                            start=(k == 0), stop=(k == n - 1),
                        )

                emit_warm_mm(4, "boundary")
                last_g = max(use_block.values())
                for sb2 in range(NB2):
                    for g in groups_at.get(sb2, []):
                        j, qt = divmod(g, NPAIR)
                        emit_pq_qt(xq_blocks[j], QS0 + j, qt, ps_q2)
                    if sb2 > last_g:
                        emit_warm_mm(2, f"tail_{sb2}")
                    # two psum tiles of 2 pairs each: 2*130 f32 = 1040 B/bank
                    nds = [
                        ps_nd.tile([P, 2, NDW], F32, tag=f"nd{i}",
                                   name=f"nd{i}_{sb2}")
                        for i in range(2)
                    ]
                    for pr in range(NPAIR):
                        nc.tensor.matmul(
                            nds[pr // 2][:, pr % 2, :],
                            phiq_sb[:, pr, sb2 * P:(sb2 + 1) * P],
                            rhs_all[:, pr, :],
                            start=(pr % 2 == 0), stop=(pr % 2 == 1),
                        )
                    # rec = 1/(den+eps). den >= 64*8192 (phi >= 1 everywhere),
                    # so EPS=1e-6 vanishes in fp32 rounding — skip the eps add,
                    # reciprocal straight from PSUM.
                    rec = bwork.tile([P, 2, 2, 2], F32, tag="rec")
                    for i in range(2):
                        nc.vector.reciprocal(rec[:, i], nds[i][:, :, P:P + 2])
                    o_t = bout.tile([P, 2 * NPAIR, HD], BF16, tag="o")
                    # broadcast multiply: out[s, h, v] = num * rec[s, h]
                    # one 4D-AP op per nd tile (2 pairs each)
                    for i in range(2):
                        nc.vector.tensor_tensor(
                            o_t[:, 4 * i:4 * i + 4, :].rearrange(
                                "p (q j) v -> p q j v", q=2
                            ),
                            nds[i][:, :, 0:P].rearrange(
                                "p q (j v) -> p q j v", v=HD
                            ),
                            rec[:, i, :, :, None].to_broadcast((P, 2, 2, HD)),
                            mybir.AluOpType.mult,
                        )
                    nc.sync.dma_start(
                        out=out[sb2 * P:(sb2 + 1) * P, :],
                        in_=o_t[:, :, :].rearrange("p h v -> p (h v)"),
                    )
    nc.finalize()
    return nc


def _get_nc():
    if "nc" not in _CACHE:
        _CACHE["nc"] = _build_nc()
    return _CACHE["nc"]


def _prep_inputs(x, W_qkv, b_qkv, W_p, b_p):
    """Host-side sharding + weight folding (fp64 fold; fp8 Q/K, bf16 V).
    Biases are zero by construction in setup_inputs(); the fold keeps the
    zero bias exact."""
    x = np.asarray(x, dtype=np.float32)
    W_qkv = np.asarray(W_qkv, dtype=np.float32)
    W_p = np.asarray(W_p, dtype=np.float32)
    bf16 = ml_dtypes.bfloat16
    f8 = ml_dtypes.float8_e4m3

    Wq = W_qkv[0:D]
    Wk = W_qkv[D:2 * D]
    Wv = W_qkv[2 * D:3 * D]
    Wp64 = W_p.astype(np.float64)

    xt_b8 = [np.ascontiguousarray(x[b].T.astype(f8)) for b in range(B)]
    xt_bf = [np.ascontiguousarray(x[b].T.astype(bf16)) for b in range(B)]

    in_maps = []
    for core in range(NCORES):
        b = core % B
        g = core // B
        rows = slice(g * CV, (g + 1) * CV)
        Wq_g = Wq[rows].astype(np.float64).reshape(HG, HD, D)
        Wk_g = Wk[rows].astype(np.float64).reshape(HG, HD, D)
        # fold the shared AD-projection into the qkv projection
        wqp_g = np.einsum("ah,ghd->gad", Wp64, Wq_g).reshape(CH, D)
        wkp_g = np.einsum("ah,ghd->gad", Wp64, Wk_g).reshape(CH, D)
        in_maps.append({
            "xt8": xt_b8[b],
            "xbf": xt_bf[b],
            "wq8": np.ascontiguousarray(wqp_g.T.astype(f8)),
            "wk8": np.ascontiguousarray(wkp_g.T.astype(f8)),
            "wv": np.ascontiguousarray(Wv[rows].T.astype(bf16)),
        })
    return in_maps


def kernel(x, W_qkv, b_qkv, W_p, b_p):
    global LAST_RESULTS
    in_maps = _prep_inputs(x, W_qkv, b_qkv, W_p, b_p)
    res = run_bass_kernel_spmd(_get_nc(), in_maps, core_ids=list(range(NCORES)))
    LAST_RESULTS = res
    out_full = np.empty((B, S, D), np.float32)
    for core in range(NCORES):
        b = core % B
        g = core // B
        out_full[b, :, g * CV:(g + 1) * CV] = \
            res.results[core]["out"].astype(np.float32)
    return out_full


# revision 24
# speedup vs baseline: 1.0064x; 1.0064x over previous
"""Linear-attention (sparse_attention) Trainium2 Bass kernel.

Problem: nn_Attention_Linear_25709674234652
  B=4, S=8192, D=1024, H=16 heads, HD=64, AD=64 (approx dim), EPS=1e-6

  qkv = x @ W_qkv.T (+0)          [B,S,3D]
  per head: pQ = Q @ W_p.T, pK = K @ W_p.T, phi(u) = sqrt(1+u^2)
  KTV = phi_K^T @ V  [AD,HD],  k_sum = sum_s phi_K
  out = (phi_Q @ KTV) / (phi_Q @ k_sum + eps)

Sharding: 8 cores = 4 batches x 2 head-groups (8 heads each). Each core is
fully independent (no collectives).

Host-side tricks:
  - W_p @ W_q and W_p @ W_k are folded into single projection matrices, so
    the device computes pQ / pK directly from x; Q and K never exist.
  - x is passed transposed (x^T) so the contraction dim D is already on
    partitions; no on-chip transposes anywhere.
  - Q/K projections run in fp8-e4m3 DoubleRow (2x PE rate): their
    quantization errors largely cancel in the num/den ratio (phi damping +
    shared-error cancellation), host-sim rel err ~5.5e-3. The V projection
    must stay bf16: V-path errors hit the output un-averaged (x-fp8 for V
    alone costs ~2.1e-2).
  - output shipped bf16 and upcast to f32 on host (halves the out-DMA
    tail in pass B).

Device structure:
  - pass A (per 512-col s-block): pQ^T feature-major via fp8 DoubleRow
    (4 k-pair matmuls) -> phi -> bf16 phi_Q kept RESIDENT in SBUF; pK
    row-major fp8 DoubleRow + V row-major bf16 -> phi(pK), V -> KTV
    accumulated over all of S in PSUM (k_sum rides along as a
    ones-column appended to V). KTV matmuls are emitted ~3 blocks late
    so the in-order PE never waits on the ACT phi chain.
  - pass B (per 128-row s-block): one N=130 matmul per head-pair against
    block-diagonal KTV + k_sum columns (num and den in one shot),
    reciprocal + broadcast multiply on DVE, store bf16. The last QSHIFT
    s-blocks' pQ matmul groups are deferred into pass B to fill its
    otherwise-idle PE (they have no KTV dependency); their x8 tiles stay
    RESIDENT in SBUF from pass A (no re-DMA). Pass B is paced by a
    three-way DVE/Scalar/PE balance at ~1.0us per block.

Measured on HW: ~303 us exec at 2.4 GHz (rel err 6.5e-3); the part
sporadically runs at ~2.0 GHz (P0 power state) where the same kernel
measures ~360 us. Baseline all-bf16 kernel was ~407 us.
"""

import numpy as np
import ml_dtypes

import concourse.bass as bass
import concourse.tile as tile
from concourse import bacc, mybir
from concourse.bass_utils import run_bass_kernel_spmd

# ---- problem dims (hardcoded per spec) ----
B, S, D = 4, 8192, 1024
H, HD, AD = 16, 64, 64
EPS = 1e-6
NCORES = 8
HG = H // 2          # heads per core = 8
CH = HG * AD         # phi channels per core = 512
CV = HG * HD         # value channels per core = 512
P = 128
NKD = D // P         # 8 contraction tiles
NKP = NKD // 2       # 4 fp8 DoubleRow contraction pair-tiles
SB = 512             # pass-A s-block
NSB = S // SB        # 16
NPAIR = CH // P      # 4 head-pairs per core
NB2 = S // P         # 64 pass-B s-blocks
F32 = mybir.dt.float32
BF16 = mybir.dt.bfloat16
F8 = mybir.dt.float8e4
DR = mybir.MatmulPerfMode.DoubleRow

_CACHE = {}
LAST_RESULTS = None  # BassKernelResults of most recent run (for profiling)


def _build_nc():
    nc = bacc.Bacc()
    AF = mybir.ActivationFunctionType

    xt8 = nc.dram_tensor("xt8", [D, S], F8, kind="ExternalInput")
    xbf = nc.dram_tensor("xbf", [D, S], BF16, kind="ExternalInput")
    wq8 = nc.dram_tensor("wq8", [D, CH], F8, kind="ExternalInput")
    wk8 = nc.dram_tensor("wk8", [D, CH], F8, kind="ExternalInput")
    wv = nc.dram_tensor("wv", [D, CV], BF16, kind="ExternalInput")
    out = nc.dram_tensor("out", [S, CV], BF16, kind="ExternalOutput")

    xt8_r = xt8.rearrange("(kd p) s -> p kd s", p=P)
    xbf_r = xbf.rearrange("(kd p) s -> p kd s", p=P)
    wq8_r = wq8.rearrange("(kd p) c -> p kd c", p=P)
    wk8_r = wk8.rearrange("(kd p) c -> p kd c", p=P)
    wv_r = wv.rearrange("(kd p) c -> p kd c", p=P)

    with tile.TileContext(nc) as tc:
        with (
            tc.tile_pool(name="singles", bufs=1) as singles,
            tc.tile_pool(name="x8load", bufs=2) as x8load,
            tc.tile_pool(name="xbload", bufs=2) as xbload,
            tc.tile_pool(name="sqp", bufs=3) as sqpool,
            tc.tile_pool(name="phikp", bufs=6) as phikpool,
            tc.tile_pool(name="vp", bufs=6) as vpool,
        ):
            # startup critical path: per-kd DMAs so the first matmul starts
            # after a few hundred KB, not the full working set
            def load_x_block(sb, persist=False):
                """fp8 pair tiles [P, 2, SB] (t=kd pair) + bf16 tiles [P, SB].

                persist: deferred-pQ blocks keep their fp8 tiles resident in
                SBUF until pass B consumes them (no re-DMA, no LDW waits)."""
                t8, tbf = [], []
                for t in range(NKP):
                    if persist:
                        x8_t = x8load.tile([P, 2, SB], F8, tag=f"xqp_{sb}_{t}",
                                           name=f"x8_{sb}_{t}", bufs=1)
                    else:
                        x8_t = x8load.tile([P, 2, SB], F8, tag=f"x8_{t}",
                                           name=f"x8_{sb}_{t}")
                    for i in range(2):
                        nc.sync.dma_start(
                            out=x8_t[:, i, :],
                            in_=xt8_r[:, 2 * t + i, sb * SB:(sb + 1) * SB],
                        )
                    t8.append(x8_t)
                for kd in range(NKD):
                    xb_t = xbload.tile([P, SB], BF16, tag=f"xb{kd}",
                                       name=f"xb_{sb}_{kd}")
                    nc.sync.dma_start(
                        out=xb_t, in_=xbf_r[:, kd, sb * SB:(sb + 1) * SB]
                    )
                    tbf.append(xb_t)
                return t8, tbf

            # startup DMA order = dependency order of the early PE work:
            # x8(sb0)+wq8 (pQ sb0) -> x8(sb1) (pQ sb1) -> wk8 (pK) ->
            # xbf(sb0)+wv (V sb0) -> xbf(sb1). The two pQ emissions give the
            # PE ~14us of work before V's operands are needed, so no warm
            # bridges are required.
            w_q8 = singles.tile([P, NKD, CH], F8)
            w_k8 = singles.tile([P, NKD, CH], F8)
            w_v = singles.tile([P, NKD, CV], BF16)
            x8_first, xbf_first = [], []
            x8_second, xbf_second = [], []
            for t in range(NKP):
                x8_t = x8load.tile([P, 2, SB], F8, tag=f"x8_{t}",
                                   name=f"x8_0_{t}")
                for i in range(2):
                    nc.sync.dma_start(out=x8_t[:, i, :],
                                      in_=xt8_r[:, 2 * t + i, 0:SB])
                    nc.sync.dma_start(out=w_q8[:, 2 * t + i],
                                      in_=wq8_r[:, 2 * t + i])
                x8_first.append(x8_t)
            for t in range(NKP):
                x8_t = x8load.tile([P, 2, SB], F8, tag=f"x8_{t}",
                                   name=f"x8_1_{t}")
                for i in range(2):
                    nc.sync.dma_start(out=x8_t[:, i, :],
                                      in_=xt8_r[:, 2 * t + i, SB:2 * SB])
                x8_second.append(x8_t)
            for kd in range(NKD):
                nc.sync.dma_start(out=w_k8[:, kd], in_=wk8_r[:, kd])
            for kd in range(NKD):
                xb_t = xbload.tile([P, SB], BF16, tag=f"xb{kd}",
                                   name=f"xb_0_{kd}")
                nc.sync.dma_start(out=xb_t, in_=xbf_r[:, kd, 0:SB])
                xbf_first.append(xb_t)
                nc.sync.dma_start(out=w_v[:, kd], in_=wv_r[:, kd])
            for kd in range(NKD):
                xb_t = xbload.tile([P, SB], BF16, tag=f"xb{kd}",
                                   name=f"xb_1_{kd}")
                nc.sync.dma_start(out=xb_t, in_=xbf_r[:, kd, SB:2 * SB])
                xbf_second.append(xb_t)
            # phi_Q^T resident: [128, 4 q-tiles, S] bf16 = 64 KiB/partition
            phiq_sb = singles.tile([P, NPAIR, S], BF16)

            with (
                tc.tile_pool(name="ps_q", bufs=2, space="PSUM") as ps_q,
                tc.tile_pool(name="ps_k", bufs=2, space="PSUM") as ps_k,
                tc.tile_pool(name="ps_v", bufs=2, space="PSUM") as ps_v,
                tc.tile_pool(name="ps_acc", bufs=1, space="PSUM") as ps_acc,
            ):
                # startup: warm matmuls on DVE-memset tiles (no DMA dep)
                # fill the initial DMA wait and spin HAM up to 2.4 GHz
                # before the first real matmul
                warm_a = singles.tile([P, P], BF16)
                nc.vector.memset(warm_a, 0.5)
                warm_b = singles.tile([P, SB], BF16)
                nc.vector.memset(warm_b, 0.5)
                wp0 = ps_q.tile([P, SB], F32, tag="pq", name="warm_start")
                for k in range(12):
                    nc.tensor.matmul(
                        wp0, warm_a, warm_b, start=(k == 0), stop=(k == 11)
                    )

                def warm_bridge(n, key):
                    # no-dep filler MMs to bridge startup DMA stalls so HAM
                    # never sees an idle window while the x/w streams land
                    wp = ps_q.tile([P, SB], F32, tag="pq", name=f"wb_{key}")
                    for k in range(n):
                        nc.tensor.matmul(
                            wp, warm_a, warm_b, start=(k == 0), stop=(k == n - 1)
                        )

                # persistent accumulators, live across the whole pass.
                # col 128 of each pair block accumulates k_sum (ones column
                # appended to V), so no separate ksum matmuls are needed.
                PV1 = P + 1
                ktv_ps_ab = [
                    ps_acc.tile([P, 2, PV1], F32, tag=f"ktv{i}", name=f"ktv{i}")
                    for i in range(2)
                ]

                pending = []

                def emit_ktv(phik_t, v_t, idx):
                    first = idx == 0
                    last = idx == 4 * NSB - 1
                    for pr in range(NPAIR):
                        # [128s x 128a].T @ [128s x 129(v|1)] -> a-pair x (v|ksum)
                        # off-diagonal 64x64 blocks are cross-head garbage,
                        # masked out when copying to SBUF.
                        nc.tensor.matmul(
                            ktv_ps_ab[pr // 2][:, pr % 2, :],
                            phik_t[:, pr * P:(pr + 1) * P],
                            v_t[:, pr, :],
                            start=(first and pr % 2 == 0),
                            stop=(last and pr % 2 == 1),
                        )

                def emit_pq_qt(x8_t, sb, qt, pool):
                    # one pQ^T q-tile: fp8 DoubleRow matmul group + phi ->
                    # resident bf16
                    pq_ps = pool.tile([P, SB], F32, tag="pq",
                                      name=f"pq_{sb}_{qt}")
                    for t in range(NKP):
                        nc.tensor.matmul(
                            pq_ps,
                            w_q8[:, 2 * t:2 * t + 2, qt * P:(qt + 1) * P],
                            x8_t[t],
                            start=(t == 0),
                            stop=(t == NKP - 1),
                            perf_mode=DR,
                        )
                    sq_t = sqpool.tile([P, SB], F32, tag="sq_q")
                    nc.scalar.square(sq_t, pq_ps)
                    nc.scalar.activation(
                        phiq_sb[:, qt, sb * SB:(sb + 1) * SB],
                        sq_t, AF.Sqrt, bias=1.0,
                    )

                def emit_pq(x8_t, sb, pool):
                    for qt in range(NPAIR):
                        emit_pq_qt(x8_t, sb, qt, pool)

                # the last QSHIFT blocks' pQ groups are deferred into pass B
                # (no KTV dependency): spread over pass B at qt-group
                # granularity to keep the PE dense there
                QSHIFT = 11
                QS0 = NSB - QSHIFT
                xq_blocks = {}
                # pQ of sb0 AND sb1 first: their deps (x8 + wq8, ~1.7 MiB)
                # land long before V's xbf/wv, keeping the PE busy through
                # the startup DMA window
                emit_pq(x8_first, 0, ps_q)
                emit_pq(x8_second, 1, ps_q)
                # startup is DMA-throughput-bound (~20us to land xbf+wv);
                # DMA-free filler keeps HAM at 2.4 GHz until V's deps arrive
                warm_bridge(6, "post_pq")
                for sb in range(NSB):
                    if sb == 0:
                        x8_t, xbf_t = x8_first, xbf_first
                    elif sb == 1:
                        x8_t, xbf_t = x8_second, xbf_second
                    else:
                        x8_t, xbf_t = load_x_block(sb, persist=(sb >= QS0))
                    if 2 <= sb < QS0:
                        emit_pq(x8_t, sb, ps_q)
                    elif sb >= QS0:
                        xq_blocks[sb - QS0] = x8_t
                    # ---- row-major pK (fp8 DR) | V (bf16) + phi + KTV ----
                    for st in range(4):
                        pk_ps = ps_k.tile([P, CH], F32, tag="pk")
                        v_ps = ps_v.tile([P, CV], F32, tag="v")
                        for t in range(NKP):
                            nc.tensor.matmul(
                                pk_ps,
                                x8_t[t][:, :, st * P:(st + 1) * P],
                                w_k8[:, 2 * t:2 * t + 2, :],
                                start=(t == 0), stop=(t == NKP - 1),
                                perf_mode=DR,
                            )
                        if sb == 0:
                            warm_bridge((3, 2, 1, 1)[st], f"s0_{st}")
                        elif sb == 1 and st == 0:
                            warm_bridge(1, "s1_0")
                        # emit deferred KTV here: the tiny KTV matmuls land
                        # between the DR pk group and the bf16 V group, so
                        # the next DR LDWEIGHTS never trails a tiny-MM burst
                        while len(pending) > 3:
                            emit_ktv(*pending.pop(0))
                        for kd in range(NKD):
                            nc.tensor.matmul(
                                v_ps,
                                xbf_t[kd][:, st * P:(st + 1) * P],
                                w_v[:, kd, :],
                                start=(kd == 0), stop=(kd == NKD - 1),
                            )
                        sqk_t = sqpool.tile([P, CH], F32, tag="sq_k")
                        nc.scalar.square(sqk_t, pk_ps)
                        phik_t = phikpool.tile([P, CH], BF16, tag="phik")
                        nc.scalar.activation(phik_t, sqk_t, AF.Sqrt, bias=1.0)
                        # V pairs with a ones column appended (k_sum rides the
                        # KTV matmul as output column 128)
                        v_t = vpool.tile([P, NPAIR, P + 1], BF16, tag="vsb")
                        nc.vector.tensor_copy(
                            out=v_t[:, :, 0:P],
                            in_=v_ps[:, :].rearrange("p (q v) -> p q v", v=P),
                        )
                        nc.vector.memset(v_t[:, :, P:P + 1], 1.0)
                        pending.append((phik_t, v_t, sb * 4 + st))
                for item in pending:
                    emit_ktv(*item)
                pending.clear()

                # ---- KTV -> block-diag SBUF (bf16), ksum in cols 128-129 ----
                # rhs_all[:, pr] = [ktv_bd (128) | ksum_h0 col | ksum_h1 col]
                # so pass B's den rides the same matmul as num (N=130).
                rhs_all = singles.tile([P, NPAIR, P + 2], BF16)
                nc.vector.memset(rhs_all, 0.0)
                HA = AD  # 64
                for pr in range(NPAIR):
                    kps = ktv_ps_ab[pr // 2][:, pr % 2, :]
                    nc.vector.tensor_copy(
                        out=rhs_all[0:HA, pr, 0:HA], in_=kps[0:HA, 0:HA]
                    )
                    nc.vector.tensor_copy(
                        out=rhs_all[HA:P, pr, HA:P], in_=kps[HA:P, HA:P]
                    )
                    nc.vector.tensor_copy(
                        out=rhs_all[0:HA, pr, P:P + 1], in_=kps[0:HA, P:P + 1]
                    )
                    nc.vector.tensor_copy(
                        out=rhs_all[HA:P, pr, P + 1:P + 2], in_=kps[HA:P, P:P + 1]
                    )

            # ---- pass B: numerator / denominator / divide / store ----
            with (
                tc.tile_pool(name="ps_nd", bufs=3, space="PSUM") as ps_nd,
                tc.tile_pool(name="ps_q2", bufs=2, space="PSUM") as ps_q2,
                tc.tile_pool(name="bwork", bufs=4) as bwork,
                tc.tile_pool(name="bout", bufs=4) as bout,
            ):
                NDW = P + 2  # num (128) + den (2) columns per pair
                NQG = NPAIR * QSHIFT   # deferred qt-groups
                # uniform deadline-aware placement: group g at use_block(g),
                # spread over nearly all of pass B so the PE (and HAM) stay
                # dense to the end. Deadline: group (j, qt) must land >=2
                # blocks before nd block 4*(QS0+j) reads its phiq. The x8
                # operands are already resident in SBUF (persist=True above).
                use_block = {}
                for g in range(NQG):
                    j = g // NPAIR
                    blk = 1 + (g * 58) // max(NQG, 1)
                    blk = min(blk, 4 * (QS0 + j) - 2)
                    use_block[g] = blk
                groups_at = {}
                for g in range(NQG):
                    groups_at.setdefault(use_block[g], []).append(g)

                def emit_warm_mm(n, key):
                    # dummy matmuls on resident weights into a dead psum tile:
                    # keeps the PE duty cycle high enough that HAM doesn't
                    # re-throttle to 1.2 GHz during bursty stretches
                    wp = ps_q2.tile([P, SB], F32, tag="pq", name=f"warm_{key}")
                    for k in range(n):
                        nc.tensor.matmul(
                            wp, w_v[:, k, 0:P], w_v[:, k, :],
                            start=(k == 0), stop=(k == n - 1),
                        )

                emit_warm_mm(4, "boundary")
                last_g = max(use_block.values())
                for sb2 in range(NB2):
                    for g in groups_at.get(sb2, []):
                        j, qt = divmod(g, NPAIR)
                        emit_pq_qt(xq_blocks[j], QS0 + j, qt, ps_q2)
                    if sb2 > last_g:
                        emit_warm_mm(2, f"tail_{sb2}")
                    # two psum tiles of 2 pairs each: 2*130 f32 = 1040 B/bank
                    nds = [
                        ps_nd.tile([P, 2, NDW], F32, tag=f"nd{i}",
                                   name=f"nd{i}_{sb2}")
                        for i in range(2)
                    ]
                    for pr in range(NPAIR):
                        nc.tensor.matmul(
                            nds[pr // 2][:, pr % 2, :],
                            phiq_sb[:, pr, sb2 * P:(sb2 + 1) * P],
                            rhs_all[:, pr, :],
                            start=(pr % 2 == 0), stop=(pr % 2 == 1),
                        )
                    # rec = 1/(den+eps). den >= 64*8192 (phi >= 1 everywhere),
                    # so EPS=1e-6 vanishes in fp32 rounding — skip the eps add,
                    # reciprocal straight from PSUM.
                    rec = bwork.tile([P, 2, 2, 2], F32, tag="rec")
                    for i in range(2):
                        nc.vector.reciprocal(rec[:, i], nds[i][:, :, P:P + 2])
                    o_t = bout.tile([P, 2 * NPAIR, HD], BF16, tag="o")
                    # broadcast multiply: out[s, h, v] = num * rec[s, h]
                    # one 4D-AP op per nd tile (2 pairs each)
                    for i in range(2):
                        nc.vector.tensor_tensor(
                            o_t[:, 4 * i:4 * i + 4, :].rearrange(
                                "p (q j) v -> p q j v", q=2
                            ),
                            nds[i][:, :, 0:P].rearrange(
                                "p q (j v) -> p q j v", v=HD
                            ),
                            rec[:, i, :, :, None].to_broadcast((P, 2, 2, HD)),
                            mybir.AluOpType.mult,
                        )
                    nc.sync.dma_start(
                        out=out[sb2 * P:(sb2 + 1) * P, :],
                        in_=o_t[:, :, :].rearrange("p h v -> p (h v)"),
                    )
    nc.finalize()
    return nc


def _get_nc():
    if "nc" not in _CACHE:
        _CACHE["nc"] = _build_nc()
    return _CACHE["nc"]


def _prep_inputs(x, W_qkv, b_qkv, W_p, b_p):
    """Host-side sharding + weight folding (fp64 fold; fp8 Q/K, bf16 V).
    Biases are zero by construction in setup_inputs(); the fold keeps the
    zero bias exact."""
    x = np.asarray(x, dtype=np.float32)
    W_qkv = np.asarray(W_qkv, dtype=np.float32)
    W_p = np.asarray(W_p, dtype=np.float32)
    bf16 = ml_dtypes.bfloat16
    f8 = ml_dtypes.float8_e4m3

    Wq = W_qkv[0:D]
    Wk = W_qkv[D:2 * D]
    Wv = W_qkv[2 * D:3 * D]
    Wp64 = W_p.astype(np.float64)

    xt_b8 = [np.ascontiguousarray(x[b].T.astype(f8)) for b in range(B)]
    xt_bf = [np.ascontiguousarray(x[b].T.astype(bf16)) for b in range(B)]

    in_maps = []
    for core in range(NCORES):
        b = core % B
        g = core // B
        rows = slice(g * CV, (g + 1) * CV)
        Wq_g = Wq[rows].astype(np.float64).reshape(HG, HD, D)
        Wk_g = Wk[rows].astype(np.float64).reshape(HG, HD, D)
        # fold the shared AD-projection into the qkv projection
        wqp_g = np.einsum("ah,ghd->gad", Wp64, Wq_g).reshape(CH, D)
        wkp_g = np.einsum("ah,ghd->gad", Wp64, Wk_g).reshape(CH, D)
        in_maps.append({
            "xt8": xt_b8[b],
            "xbf": xt_bf[b],
            "wq8": np.ascontiguousarray(wqp_g.T.astype(f8)),
            "wk8": np.ascontiguousarray(wkp_g.T.astype(f8)),
            "wv": np.ascontiguousarray(Wv[rows].T.astype(bf16)),
        })
    return in_maps


def kernel(x, W_qkv, b_qkv, W_p, b_p):
    global LAST_RESULTS
    in_maps = _prep_inputs(x, W_qkv, b_qkv, W_p, b_p)
    res = run_bass_kernel_spmd(_get_nc(), in_maps, core_ids=list(range(NCORES)))
    LAST_RESULTS = res
    out_full = np.empty((B, S, D), np.float32)
    for core in range(NCORES):
        b = core % B
        g = core // B
        out_full[b, :, g * CV:(g + 1) * CV] = \
            res.results[core]["out"].astype(np.float32)
    return out_full
